# revision 8
# baseline (speedup 1.0000x reference)
"""TRN2 Bass kernel for nn_Attention_20633022890922.

The reference module's einsum 'bqhk,bvhd->bqhd' contracts the attention-weight
head axis (k) and the value head axis (v) independently, so the product
factorizes into (sum_k softmax(...)) * (sum_v V) = 1 * Vsum.  The whole module
is therefore algebraically a single linear layer:

    out = tokens @ (Wv_sum @ Wo_sum) + bo
      Wv_sum[h, d]  = sum_v Wv[h, v*64 + d]          (512 x 64)
      Wo_sum[d, e]  = sum_q Wo[q*64 + d, e]          (64 x 512)

(The only approximation is softmax summing to 1.0, which holds to ~1e-7 in
fp32.)  Wq / Wk cancel entirely.

Device strategy: data-parallel over the batch dim (8 batches -> 8 cores).
Per core: Y = X @ Wv_sum @ [Wo_sum; bo] with X [8192, 512] fp32.
The PE contracts over partitions, so X tiles are transposed on the tensor
engine (fp32r pass-through), then two small GEMMs produce token-major output:
  GEMM1: T^T[64, t]  = Wv_sum.T @ X^T      (K=512 via 4 accum matmuls)
  GEMM2: Y[t, 512]   = [T | 1] @ [Wo_sum; bo]   (K=65, bias folded in)
All matmuls use float32r (full PE rate at N=512; on TRN2 hardware fp32 and
fp32r matmul are bit-identical, ~12-bit operand mantissa).
"""

import numpy as np

from concourse import bacc, mybir, tile
from concourse import bass_utils

B, N_TOK, HID, EMB, NH, HD = 8, 8192, 512, 512, 8, 64
N_CORES = 8
CH = 512                      # tokens per compute chunk
NCHUNK = N_TOK // CH          # 16

F32R = mybir.dt.float32r
F32 = mybir.dt.float32

_compiled = None


def _build():
    nc = bacc.Bacc(
        trn_type="TRN2", target_bir_lowering=False, debug=False, num_devices=N_CORES
    )

    x_d = nc.dram_tensor("x", [N_TOK, HID], F32R, kind="ExternalInput")
    wv_d = nc.dram_tensor("wv", [HID, HD], F32R, kind="ExternalInput")   # Wv_sum
    wo_d = nc.dram_tensor("wo", [HD + 1, HID], F32R, kind="ExternalInput")  # [Wo_sum; bo]
    id_d = nc.dram_tensor("ident", [128, 128], F32R, kind="ExternalInput")
    ones_d = nc.dram_tensor("ones", [1, 128], F32R, kind="ExternalInput")
    y_d = nc.dram_tensor("y", [N_TOK, HID], F32R, kind="ExternalOutput")

    with tile.TileContext(nc) as tc:
        with (
            tc.tile_pool(name="const", bufs=1) as constp,
            tc.tile_pool(name="xin", bufs=3) as xin_p,
            tc.tile_pool(name="xt", bufs=10) as xt_p,
            tc.tile_pool(name="tt", bufs=3) as tt_p,
            tc.tile_pool(name="yout", bufs=3) as y_p,
            tc.tile_pool(name="ps_xt", bufs=4, space="PSUM") as ps_xt,
            tc.tile_pool(name="ps_t", bufs=2, space="PSUM") as ps_t,
            tc.tile_pool(name="ps_y", bufs=2, space="PSUM") as ps_y,
        ):
            ident = constp.tile([128, 128], F32R, tag="ident")
            nc.sync.dma_start(ident[:], id_d[:])
            wv = constp.tile([128, 4, HD], F32R, tag="wv")
            nc.sync.dma_start(wv[:], wv_d.rearrange("(j k) m -> k j m", k=128))
            wo = constp.tile([HD + 1, HID], F32R, tag="wo")
            nc.sync.dma_start(wo[:], wo_d[:])
            ones1 = constp.tile([1, 128], F32R, tag="ones")
            nc.sync.dma_start(ones1[:], ones_d[:])
            wob = constp.tile([1, HID], F32R, tag="wob")
            nc.sync.dma_start(wob[:], wo_d[HD:HD + 1, :])

            for c in range(NCHUNK):
                # ---- load X chunk [512 tokens, 512 hid], token-major
                xin = xin_p.tile([128, 4, HID], F32R, tag="xin")
                nc.sync.dma_start(
                    xin[:],
                    x_d[c * CH:(c + 1) * CH, :].rearrange("(i p) h -> p i h", p=128),
                )

                # ---- transpose to [hid, tokens] on the PE, 128x128 tiles
                xts = []
                for j in range(4):            # hid block
                    pxt = ps_xt.tile([128, CH], F32R, tag="pxt")
                    for i in range(4):        # token tile
                        nc.tensor.matmul(
                            pxt[:, 128 * i:128 * (i + 1)],
                            xin[:, i, 128 * j:128 * (j + 1)],
                            ident[:],
                            is_transpose=True,
                            start=(i == 0),
                            stop=(i == 3),
                        )
                    sxt = xt_p.tile([128, CH], F32R, tag="sxt")
                    if j % 2 == 0:
                        nc.vector.tensor_copy(sxt[:], pxt[:])
                    else:
                        nc.scalar.copy(sxt[:], pxt[:])
                    xts.append(sxt)

                # ---- GEMM1: T^T [64, 512] = Wv_sum.T @ X^T (accumulate over hid blocks)
                pt = ps_t.tile([HD, CH], F32, tag="pt")
                for j in range(4):
                    nc.tensor.matmul(
                        pt[:], wv[:, j, :], xts[j][:], start=(j == 0), stop=(j == 3)
                    )

                # ---- T^T to SBUF (fp32r-rounded for GEMM2)
                tt = tt_p.tile([HD, CH], F32R, tag="tt")
                nc.vector.tensor_copy(tt[:], pt[:])

                # ---- GEMM2: Y tile [128, 512] = T @ Wo_sum + 1 x bo
                yo = y_p.tile([128, 4, HID], F32R, tag="yo")
                for i in range(4):
                    py = ps_y.tile([128, HID], F32, tag="py")
                    nc.tensor.matmul(
                        py[:], tt[:, 128 * i:128 * (i + 1)], wo[0:HD, :],
                        start=True, stop=False,
                    )
                    nc.tensor.matmul(
                        py[:], ones1[:], wob[:],
                        start=False, stop=True,
                    )
                    if i % 2 == 0:
                        nc.vector.tensor_copy(yo[:, i, :].bitcast(F32), py[:])
                    else:
                        nc.scalar.copy(yo[:, i, :].bitcast(F32), py[:])

                # ---- store
                nc.scalar.dma_start(
                    y_d[c * CH:(c + 1) * CH, :].rearrange("(i p) h -> p i h", p=128),
                    yo[:],
                )

    nc.compile()
    return nc


def _get_compiled():
    global _compiled
    if _compiled is None:
        _compiled = _build()
    return _compiled


def kernel(tokens, Wq, Wk, Wv, Wo, bo, _trace=False):
    tokens = np.ascontiguousarray(np.asarray(tokens, dtype=np.float32))
    Wv = np.asarray(Wv, dtype=np.float32)
    Wo = np.asarray(Wo, dtype=np.float32)
    bo = np.asarray(bo, dtype=np.float32)

    # Host-side weight folding (tiny): Wv_sum [512, 64], [Wo_sum; bo] [65, 512]
    wv_sum = Wv.reshape(HID, NH, HD).sum(axis=1).astype(np.float32)
    wo_sum = Wo.reshape(NH, HD, HID).sum(axis=0).astype(np.float32)
    wo_b = np.ascontiguousarray(
        np.vstack([wo_sum, bo.reshape(1, HID)]).astype(np.float32)
    )
    ident = np.eye(128, dtype=np.float32)
    ones = np.ones((1, 128), dtype=np.float32)

    nc = _get_compiled()
    in_maps = [
        {"x": tokens[b], "wv": wv_sum, "wo": wo_b, "ident": ident, "ones": ones}
        for b in range(N_CORES)
    ]
    res = bass_utils.run_bass_kernel_spmd(
        nc, in_maps, core_ids=list(range(N_CORES)), trace=_trace
    )
    out = np.stack([res.results[b]["y"] for b in range(N_CORES)], axis=0)
    if _trace:
        return out, res
    return out


if __name__ == "__main__":
    rng = np.random.default_rng(0)
    ins = {
        "tokens": rng.standard_normal((B, N_TOK, HID)).astype(np.float32),
        "Wq": (rng.standard_normal((HID, EMB)) * 0.02).astype(np.float32),
        "Wk": (rng.standard_normal((HID, EMB)) * 0.02).astype(np.float32),
        "Wv": (rng.standard_normal((HID, HID)) * 0.02).astype(np.float32),
        "Wo": (rng.standard_normal((EMB, HID)) * 0.02).astype(np.float32),
        "bo": np.zeros((HID,), dtype=np.float32),
    }
    out = kernel(**ins)
    print(out.shape, out.dtype)


# revision 9
# speedup vs baseline: 1.0459x; 1.0459x over previous
"""TRN2 Bass kernel for nn_Attention_20633022890922.

The reference module's einsum 'bqhk,bvhd->bqhd' contracts the attention-weight
head axis (k) and the value head axis (v) independently, so the product
factorizes into (sum_k softmax(...)) * (sum_v V) = 1 * Vsum.  The whole module
is therefore algebraically a single linear layer:

    out = tokens @ Wv_sum @ Wo_sum + bo
      Wv_sum[h, d]  = sum_v Wv[h, v*64 + d]          (512 x 64)
      Wo_sum[d, e]  = sum_q Wo[q*64 + d, e]          (64 x 512)

(The only approximation is softmax summing to 1.0, which holds to ~1e-7 in
fp32.)  Wq / Wk cancel entirely.

Device strategy: data-parallel over the batch dim (8 batches -> 8 cores).
Per core: Y = X @ Wv_sum @ Wo_sum + bo with X [8192, 512].

The PE contracts over the partition dim, so X must be presented hid-major.
fp32 PE transposes serialize on their 4-byte weight loads (measured: ~2x
slowdown), so instead X is split on the host into an exact bf16 hi/lo pair
(hi + lo carries 16+ mantissa bits; TRN2's PE rounds matmul operands to ~12
bits anyway, measured) and the 16-bit DMA-transpose xbar path loads both
halves directly in [hid, token] layout.  Then:

  GEMM1 (bf16, 12 accum matmuls): T^T = Wv_hi.T@Xhi^T + Wv_hi.T@Xlo^T
                                        + Wv_lo.T@Xhi^T          [64, 512]
  GEMM2 (fp32r, 4 matmuls):       Y[t,:] = T @ Wo_sum            [128, 512]
  bias: added by the DVE during the PSUM->SBUF output copy.

Measured accuracy ~2.5e-4 max-rel — the hardware floor for any PE matmul
(fp32/fp32r matmul quantizes operands to ~12 bits; bit-identical results).
"""

import numpy as np
import ml_dtypes

from concourse import bacc, mybir, tile
from concourse import bass_utils

B, N_TOK, HID, EMB, NH, HD = 8, 8192, 512, 512, 8, 64
N_CORES = 8
CH = 512                      # tokens per compute chunk
WAVE = 2048                   # tokens per transposed-DMA wave
NCHUNK = N_TOK // CH          # 16
NWAVE = N_TOK // WAVE         # 4
CPW = WAVE // CH              # chunks per wave = 4

F32R = mybir.dt.float32r
F32 = mybir.dt.float32
BF16 = mybir.dt.bfloat16

_compiled = None


def _build():
    nc = bacc.Bacc(
        trn_type="TRN2", target_bir_lowering=False, debug=False, num_devices=N_CORES
    )

    # h-block-major bf16 halves of X: [4 hid-blocks, 8192 tokens, 128 hid]
    xhi_d = nc.dram_tensor("xhi", [4, N_TOK, 128], BF16, kind="ExternalInput")
    xlo_d = nc.dram_tensor("xlo", [4, N_TOK, 128], BF16, kind="ExternalInput")
    wvh_d = nc.dram_tensor("wvh", [HID, HD], BF16, kind="ExternalInput")
    wvl_d = nc.dram_tensor("wvl", [HID, HD], BF16, kind="ExternalInput")
    wo_d = nc.dram_tensor("wo", [HD, HID], F32R, kind="ExternalInput")
    bob_d = nc.dram_tensor("bob", [128, HID], F32, kind="ExternalInput")
    y_d = nc.dram_tensor("y", [N_TOK, HID], F32R, kind="ExternalOutput")

    with tile.TileContext(nc) as tc:
        with (
            tc.tile_pool(name="const", bufs=1) as constp,
            tc.tile_pool(name="xt", bufs=16) as xt_p,
            tc.tile_pool(name="tt", bufs=3) as tt_p,
            tc.tile_pool(name="yout", bufs=4) as y_p,
            tc.tile_pool(name="ps_t", bufs=2, space="PSUM") as ps_t,
            tc.tile_pool(name="ps_y", bufs=4, space="PSUM") as ps_y,
        ):
            wvh = constp.tile([128, 4, HD], BF16, tag="wvh")
            nc.sync.dma_start(wvh[:], wvh_d.rearrange("(j k) m -> k j m", k=128))
            wvl = constp.tile([128, 4, HD], BF16, tag="wvl")
            nc.sync.dma_start(wvl[:], wvl_d.rearrange("(j k) m -> k j m", k=128))
            wo = constp.tile([HD, HID], F32R, tag="wo")
            nc.sync.dma_start(wo[:], wo_d[:])
            bob = constp.tile([128, HID], F32, tag="bob")
            nc.sync.dma_start(bob[:], bob_d[:])

            for w in range(NWAVE):
                # ---- transposed loads: [hid 128, WAVE tokens] per hid-block/half
                xth, xtl = [], []
                for j in range(4):
                    th = xt_p.tile([128, WAVE], BF16, tag="xt")
                    nc.sync.dma_start_transpose(
                        th[:], xhi_d[j, w * WAVE:(w + 1) * WAVE, :]
                    )
                    xth.append(th)
                    tl = xt_p.tile([128, WAVE], BF16, tag="xt")
                    nc.sync.dma_start_transpose(
                        tl[:], xlo_d[j, w * WAVE:(w + 1) * WAVE, :]
                    )
                    xtl.append(tl)

                for q in range(CPW):
                    sl = slice(q * CH, (q + 1) * CH)
                    # ---- GEMM1: T^T [64, 512], 12 accumulating bf16 matmuls
                    pt = ps_t.tile([HD, CH], F32, tag="pt")
                    terms = [(wvh, xth), (wvh, xtl), (wvl, xth)]
                    n = 0
                    for ws, xs in terms:
                        for j in range(4):
                            nc.tensor.matmul(
                                pt[:], ws[:, j, :], xs[j][:, sl],
                                start=(n == 0), stop=(n == 11),
                            )
                            n += 1

                    # ---- T^T to SBUF (fp32r-rounded for GEMM2)
                    tt = tt_p.tile([HD, CH], F32R, tag="tt")
                    nc.vector.tensor_copy(tt[:], pt[:])

                    # ---- GEMM2 + bias + store
                    yo = y_p.tile([128, 4, HID], F32R, tag="yo")
                    for i in range(4):
                        py = ps_y.tile([128, HID], F32, tag="py")
                        nc.tensor.matmul(
                            py[:], tt[:, 128 * i:128 * (i + 1)], wo[:],
                            start=True, stop=True,
                        )
                        nc.vector.tensor_add(
                            yo[:, i, :].bitcast(F32), py[:], bob[:]
                        )

                    c = w * CPW + q
                    nc.scalar.dma_start(
                        y_d[c * CH:(c + 1) * CH, :].rearrange(
                            "(i p) h -> p i h", p=128
                        ),
                        yo[:],
                    )

    nc.compile()
    return nc


def _get_compiled():
    global _compiled
    if _compiled is None:
        _compiled = _build()
    return _compiled


def _split_hi_lo(x):
    hi = x.astype(ml_dtypes.bfloat16)
    lo = (x - hi.astype(np.float32)).astype(ml_dtypes.bfloat16)
    return hi, lo


def kernel(tokens, Wq, Wk, Wv, Wo, bo, _trace=False):
    tokens = np.asarray(tokens, dtype=np.float32)
    Wv = np.asarray(Wv, dtype=np.float32)
    Wo = np.asarray(Wo, dtype=np.float32)
    bo = np.asarray(bo, dtype=np.float32)

    # Host-side prep: fold weights, split X into exact bf16 hi/lo halves,
    # reorder to hid-block-major for contiguous xbar-transpose reads.
    wv_sum = Wv.reshape(HID, NH, HD).sum(axis=1).astype(np.float32)
    wo_sum = np.ascontiguousarray(
        Wo.reshape(NH, HD, HID).sum(axis=0).astype(np.float32)
    )
    wvh, wvl = _split_hi_lo(wv_sum)
    bob = np.ascontiguousarray(np.broadcast_to(bo, (128, HID))).astype(np.float32)

    xhi, xlo = _split_hi_lo(tokens)          # [B, N, 512] bf16 each
    # -> [B, 4, N, 128] h-block-major
    xhi = np.ascontiguousarray(xhi.reshape(B, N_TOK, 4, 128).transpose(0, 2, 1, 3))
    xlo = np.ascontiguousarray(xlo.reshape(B, N_TOK, 4, 128).transpose(0, 2, 1, 3))

    nc = _get_compiled()
    in_maps = [
        {"xhi": xhi[b], "xlo": xlo[b], "wvh": wvh, "wvl": wvl,
         "wo": wo_sum, "bob": bob}
        for b in range(N_CORES)
    ]
    res = bass_utils.run_bass_kernel_spmd(
        nc, in_maps, core_ids=list(range(N_CORES)), trace=_trace
    )
    out = np.stack([res.results[b]["y"] for b in range(N_CORES)], axis=0)
    if _trace:
        return out, res
    return out


if __name__ == "__main__":
    rng = np.random.default_rng(0)
    ins = {
        "tokens": rng.standard_normal((B, N_TOK, HID)).astype(np.float32),
        "Wq": (rng.standard_normal((HID, EMB)) * 0.02).astype(np.float32),
        "Wk": (rng.standard_normal((HID, EMB)) * 0.02).astype(np.float32),
        "Wv": (rng.standard_normal((HID, HID)) * 0.02).astype(np.float32),
        "Wo": (rng.standard_normal((EMB, HID)) * 0.02).astype(np.float32),
        "bo": np.zeros((HID,), dtype=np.float32),
    }
    out = kernel(**ins)
    print(out.shape, out.dtype)


# revision 10
# speedup vs baseline: 1.4135x; 1.3515x over previous
"""TRN2 Bass kernel for nn_Attention_20633022890922.

The reference module's einsum 'bqhk,bvhd->bqhd' contracts the attention-weight
head axis (k) and the value head axis (v) independently, so the product
factorizes into (sum_k softmax(...)) * (sum_v V) = 1 * Vsum.  The whole module
is therefore algebraically a single linear layer:

    out = tokens @ Wv_sum @ Wo_sum + bo
      Wv_sum[h, d]  = sum_v Wv[h, v*64 + d]          (512 x 64)
      Wo_sum[d, e]  = sum_q Wo[q*64 + d, e]          (64 x 512)

(The only approximation is softmax summing to 1.0, which holds to ~1e-7 in
fp32.)  Wq / Wk cancel entirely.

Device strategy: data-parallel over the batch dim (8 batches -> 8 cores).
Per core: Y = X @ Wv_sum @ Wo_sum + bo with X [8192, 512].

The PE contracts over the partition dim, so X must be presented hid-major.
Measured on TRN2: the PE quantizes matmul operands to ~12 mantissa bits no
matter the input dtype (fp32 and fp32r matmuls are bit-identical), so
shipping X as fp16 (11-bit significand) costs almost nothing in accuracy
(host-emulated: 2.6e-4 vs 2.1e-4 max-rel) while halving input DMA bytes and
enabling the 16-bit DMA-transpose xbar path, which loads X directly in
[hid, token] layout -- no PE transposes, no PSUM round-trip for X.

  GEMM1 (fp16, 8 accum matmuls / 512 tokens):
        T^T = (Wv_hi + Wv_lo).T @ X^T        [64, 512]
        (Wv as an exact fp16 hi/lo pair keeps the weight side lossless)
  GEMM2 (fp32r, 4 matmuls): Y[t, :] = T @ Wo_sum   [128, 512] per t-tile
  bias: added by the DVE during the PSUM->SBUF output copy.
"""

import numpy as np

from concourse import bacc, mybir, tile
from concourse import bass_utils

B, N_TOK, HID, EMB, NH, HD = 8, 8192, 512, 512, 8, 64
N_CORES = 8
CH = 512                      # tokens per compute chunk
WAVE = 2048                   # tokens per transposed-DMA wave
NCHUNK = N_TOK // CH          # 16
NWAVE = N_TOK // WAVE         # 4
CPW = WAVE // CH              # chunks per wave = 4

F32R = mybir.dt.float32r
F32 = mybir.dt.float32
FP16 = mybir.dt.float16

_compiled = None


def _build():
    nc = bacc.Bacc(
        trn_type="TRN2", target_bir_lowering=False, debug=False, num_devices=N_CORES
    )

    # h-block-major fp16 X: [4 hid-blocks, 8192 tokens, 128 hid]
    xf_d = nc.dram_tensor("xf", [4, N_TOK, 128], FP16, kind="ExternalInput")
    wvh_d = nc.dram_tensor("wvh", [HID, HD], FP16, kind="ExternalInput")
    wvl_d = nc.dram_tensor("wvl", [HID, HD], FP16, kind="ExternalInput")
    wo_d = nc.dram_tensor("wo", [HD, HID], F32R, kind="ExternalInput")
    bob_d = nc.dram_tensor("bob", [128, HID], F32, kind="ExternalInput")
    y_d = nc.dram_tensor("y", [N_TOK, HID], F32R, kind="ExternalOutput")

    with tile.TileContext(nc) as tc:
        with (
            tc.tile_pool(name="const", bufs=1) as constp,
            tc.tile_pool(name="xt", bufs=12) as xt_p,
            tc.tile_pool(name="tt", bufs=3) as tt_p,
            tc.tile_pool(name="yout", bufs=4) as y_p,
            tc.tile_pool(name="ps_t", bufs=2, space="PSUM") as ps_t,
            tc.tile_pool(name="ps_y", bufs=4, space="PSUM") as ps_y,
        ):
            wvh = constp.tile([128, 4, HD], FP16, tag="wvh")
            nc.sync.dma_start(wvh[:], wvh_d.rearrange("(j k) m -> k j m", k=128))
            wvl = constp.tile([128, 4, HD], FP16, tag="wvl")
            nc.sync.dma_start(wvl[:], wvl_d.rearrange("(j k) m -> k j m", k=128))
            wo = constp.tile([HD, HID], F32R, tag="wo")
            nc.sync.dma_start(wo[:], wo_d[:])
            bob = constp.tile([128, HID], F32, tag="bob")
            nc.sync.dma_start(bob[:], bob_d[:])

            for w in range(NWAVE):
                # ---- transposed loads: [hid 128, WAVE tokens] per hid-block
                xt = []
                for j in range(4):
                    t = xt_p.tile([128, WAVE], FP16, tag="xt")
                    nc.sync.dma_start_transpose(
                        t[:], xf_d[j, w * WAVE:(w + 1) * WAVE, :]
                    )
                    xt.append(t)

                for q in range(CPW):
                    sl = slice(q * CH, (q + 1) * CH)
                    # ---- GEMM1: T^T [64, 512], 8 accumulating fp16 matmuls
                    pt = ps_t.tile([HD, CH], F32, tag="pt")
                    n = 0
                    for ws in (wvh, wvl):
                        for j in range(4):
                            nc.tensor.matmul(
                                pt[:], ws[:, j, :], xt[j][:, sl],
                                start=(n == 0), stop=(n == 7),
                            )
                            n += 1

                    # ---- T^T to SBUF (fp32r-rounded for GEMM2)
                    tt = tt_p.tile([HD, CH], F32R, tag="tt")
                    nc.vector.tensor_copy(tt[:], pt[:])

                    # ---- GEMM2 + bias + store
                    yo = y_p.tile([128, 4, HID], F32R, tag="yo")
                    for i in range(4):
                        py = ps_y.tile([128, HID], F32, tag="py")
                        nc.tensor.matmul(
                            py[:], tt[:, 128 * i:128 * (i + 1)], wo[:],
                            start=True, stop=True,
                        )
                        nc.vector.tensor_add(
                            yo[:, i, :].bitcast(F32), py[:], bob[:]
                        )

                    c = w * CPW + q
                    nc.scalar.dma_start(
                        y_d[c * CH:(c + 1) * CH, :].rearrange(
                            "(i p) h -> p i h", p=128
                        ),
                        yo[:],
                    )

    nc.compile()
    return nc


def _get_compiled():
    global _compiled
    if _compiled is None:
        _compiled = _build()
    return _compiled


def kernel(tokens, Wq, Wk, Wv, Wo, bo, _trace=False):
    tokens = np.asarray(tokens, dtype=np.float32)
    Wv = np.asarray(Wv, dtype=np.float32)
    Wo = np.asarray(Wo, dtype=np.float32)
    bo = np.asarray(bo, dtype=np.float32)

    # Host-side prep: fold weights (exact fp16 hi/lo pair for Wv_sum), cast X
    # to fp16, reorder to hid-block-major for contiguous xbar-transpose reads.
    wv_sum = Wv.reshape(HID, NH, HD).sum(axis=1).astype(np.float32)
    wo_sum = np.ascontiguousarray(
        Wo.reshape(NH, HD, HID).sum(axis=0).astype(np.float32)
    )
    wvh = wv_sum.astype(np.float16)
    wvl = (wv_sum - wvh.astype(np.float32)).astype(np.float16)
    bob = np.ascontiguousarray(np.broadcast_to(bo, (128, HID))).astype(np.float32)

    xf = tokens.astype(np.float16)           # [B, N, 512]
    xf = np.ascontiguousarray(xf.reshape(B, N_TOK, 4, 128).transpose(0, 2, 1, 3))

    nc = _get_compiled()
    in_maps = [
        {"xf": xf[b], "wvh": wvh, "wvl": wvl, "wo": wo_sum, "bob": bob}
        for b in range(N_CORES)
    ]
    res = bass_utils.run_bass_kernel_spmd(
        nc, in_maps, core_ids=list(range(N_CORES)), trace=_trace
    )
    out = np.stack([res.results[b]["y"] for b in range(N_CORES)], axis=0)
    if _trace:
        return out, res
    return out


if __name__ == "__main__":
    rng = np.random.default_rng(0)
    ins = {
        "tokens": rng.standard_normal((B, N_TOK, HID)).astype(np.float32),
        "Wq": (rng.standard_normal((HID, EMB)) * 0.02).astype(np.float32),
        "Wk": (rng.standard_normal((HID, EMB)) * 0.02).astype(np.float32),
        "Wv": (rng.standard_normal((HID, HID)) * 0.02).astype(np.float32),
        "Wo": (rng.standard_normal((EMB, HID)) * 0.02).astype(np.float32),
        "bo": np.zeros((HID,), dtype=np.float32),
    }
    out = kernel(**ins)
    print(out.shape, out.dtype)
